# revision 6
# baseline (speedup 1.0000x reference)
"""Trainium2 Bass kernel for nn_BERTMADActQuantizer.

y = (clip(round(x / d[grp]) + zp[grp], 0, 255) - zp[grp]) * d[grp]
where grp = clip(#(medians <= |x|), 0, 9).

Data-parallel shard of x across 8 NeuronCores (shapes hardcoded). Per core a
raw-bass double-buffered pipeline streams [128, F] tiles: SYNC does the HBM
DMAs, ACT computes |x|, DVE applies the 10-group quantizer with all-scalar
constants (clamp-first formulation + magic-number round-half-even) and merges
group results with predicated copies keyed on exact fp32 |x| >= median
comparisons.
"""

import sys

for _p in ("/opt/trn_rl_repo", "/root/.axon_site/_ro/trn_rl_repo"):
    if _p not in sys.path:
        sys.path.append(_p)

import numpy as np

from concourse import bass, mybir

N_CORES = 8
FULL_SHAPE = (4, 4096, 4096)
TOTAL = FULL_SHAPE[0] * FULL_SHAPE[1] * FULL_SHAPE[2]  # 67108864
SHARD = TOTAL // N_CORES  # 8388608
P = 128
F = 2048
TILES = SHARD // (P * F)  # 32
G = 10
MAGIC = float(np.float32(1.5 * 2**23))  # add+sub rounds to nearest int (RNE)

f32 = mybir.dt.float32
u8 = mybir.dt.uint8
Alu = mybir.AluOpType


def _f32(v) -> float:
    return float(np.float32(v))


def build_program(medians, deltas, zero_points, tiles=TILES):
    med = np.asarray(medians, dtype=np.float32)
    d = np.asarray(deltas, dtype=np.float32)
    zp = np.asarray(zero_points, dtype=np.float32)

    r = (np.float32(1.0) / d).astype(np.float32)
    A = (-zp * d).astype(np.float32)
    B = ((np.float32(255.0) - zp) * d).astype(np.float32)

    nc = bass.Bass()
    xin = nc.declare_dram_parameter("x", [tiles, P, F], f32, isOutput=False)
    yout = nc.declare_dram_parameter("y", [tiles, P, F], f32, isOutput=True)

    with (
        nc.Block() as block,
        nc.semaphore("s_ld0") as s_ld0,  # +16 per even-tile input DMA
        nc.semaphore("s_ld1") as s_ld1,  # +16 per odd-tile input DMA
        nc.semaphore("s_ab") as s_ab,  # +1 per |x| tile (ACT)
        nc.semaphore("s_dv") as s_dv,  # +1 per finished output tile (DVE)
        nc.semaphore("s_st0") as s_st0,  # +16 per even-tile output DMA
        nc.semaphore("s_st1") as s_st1,  # +16 per odd-tile output DMA
        nc.sbuf_tensor("xt", [P, 2, F], f32) as xt,
        nc.sbuf_tensor("tb", [P, 2, F], f32) as tb,
        nc.sbuf_tensor("yy", [P, 2, F], f32) as yy,
        nc.sbuf_tensor("ww", [P, 2, F], f32) as ww,
        nc.sbuf_tensor("yg", [P, 2, F], f32) as yg,
        nc.sbuf_tensor("mk", [P, 2, F], u8) as mk,
    ):
        s_ld = (s_ld0, s_ld1)
        s_st = (s_st0, s_st1)

        def ld_val(t):  # value of s_ld[t%2] once tile t's load landed
            return 16 * (t // 2 + 1)

        def st_val(t):  # value of s_st[t%2] once tile t's store landed
            return 16 * (t // 2 + 1)

        @block.sync
        def _(sync: bass.BassEngine):
            for t in range(tiles):
                s = t % 2
                if t >= 2:
                    # xt[:, s] reusable once ACT(|x|) and DVE of tile t-2 are done
                    sync.wait_ge(s_ab, t - 1)
                    sync.wait_ge(s_dv, t - 1)
                sync.dma_start(out=xt[:, s], in_=xin[t]).then_inc(s_ld[s], 16)
                if t >= 1:
                    # store tile t-1 once DVE finished it
                    sync.wait_ge(s_dv, t)
                    sync.dma_start(out=yout[t - 1], in_=yy[:, (t - 1) % 2]).then_inc(
                        s_st[(t - 1) % 2], 16
                    )
            t = tiles - 1
            sync.wait_ge(s_dv, tiles)
            sync.dma_start(out=yout[t], in_=yy[:, t % 2]).then_inc(s_st[t % 2], 16)
            sync.wait_ge(s_st0, st_val(2 * ((tiles - 1) // 2)))
            sync.wait_ge(s_st1, st_val(2 * ((tiles - 2) // 2) + 1))

        @block.scalar
        def _(scalar: bass.BassEngine):
            for t in range(tiles):
                s = t % 2
                scalar.wait_ge(s_ld[s], ld_val(t))
                if t >= 2:
                    scalar.wait_ge(s_dv, t - 1)  # DVE done reading tb[:, s]
                scalar.activation(
                    out=tb[:, s], in_=xt[:, s], func=mybir.ActivationFunctionType.Abs
                ).then_inc(s_ab, 1)

        @block.vector
        def _(vector: bass.BassEngine):
            for t in range(tiles):
                s = t % 2
                vector.wait_ge(s_ld[s], ld_val(t))
                if t >= 2:
                    vector.wait_ge(s_st[s], st_val(t - 2))  # yy[:, s] stored out
                vector.wait_ge(s_ab, t + 1)
                for g in range(G):
                    dst = yy[:, s] if g == 0 else yg[:, s]
                    # w = clamp(x, A_g, B_g)
                    vector.tensor_scalar(
                        out=ww[:, s], in0=xt[:, s],
                        scalar1=_f32(A[g]), scalar2=_f32(B[g]),
                        op0=Alu.max, op1=Alu.min,
                    )
                    vector.drain()
                    # w = w * r_g + MAGIC  (RNE round to integer grid)
                    vector.tensor_scalar(
                        out=ww[:, s], in0=ww[:, s],
                        scalar1=_f32(r[g]), scalar2=MAGIC,
                        op0=Alu.mult, op1=Alu.add,
                    )
                    vector.drain()
                    # dst = (w - MAGIC) * d_g
                    vector.tensor_scalar(
                        out=dst, in0=ww[:, s],
                        scalar1=MAGIC, scalar2=_f32(d[g]),
                        op0=Alu.subtract, op1=Alu.mult,
                    )
                    vector.drain()
                    if g > 0:
                        vector.tensor_scalar(
                            out=mk[:, s], in0=tb[:, s],
                            scalar1=_f32(med[g - 1]), scalar2=None,
                            op0=Alu.is_ge,
                        )
                        vector.drain()
                        vector.copy_predicated(out=yy[:, s], mask=mk[:, s], data=yg[:, s])
                        vector.drain()
                vector.nop().then_inc(s_dv, 1)

    return nc


def run(x, medians, deltas, zero_points, trace=False):
    from concourse.bass_utils import run_bass_kernel_spmd

    nc = build_program(medians, deltas, zero_points)

    xf = np.ascontiguousarray(np.asarray(x, dtype=np.float32)).reshape(-1)
    shards = [
        xf[i * SHARD : (i + 1) * SHARD].reshape(TILES, P, F) for i in range(N_CORES)
    ]
    in_maps = [{"x": s} for s in shards]
    res = run_bass_kernel_spmd(nc, in_maps, list(range(N_CORES)), trace=trace)
    out = np.concatenate(
        [res.results[i]["y"].reshape(-1) for i in range(N_CORES)]
    ).reshape(FULL_SHAPE)
    return out.astype(np.float32), res


def kernel(x, medians, deltas, zero_points):
    out, _ = run(x, medians, deltas, zero_points, trace=False)
    return out


# revision 12
# speedup vs baseline: 1.1374x; 1.1374x over previous
"""Trainium2 Bass kernel for nn_BERTMADActQuantizer.

y = (clip(round(x / d[grp]) + zp[grp], 0, 255) - zp[grp]) * d[grp]
where grp = clip(#(medians <= |x|), 0, 9).

Data-parallel shard of x across 8 NeuronCores (shapes hardcoded). Per core a
raw-bass double-buffered pipeline streams [128, F] tiles. Work is split
between the two elementwise engines:

  ACT: |x|; per group g the RNE round t2 = fma(w, 1/d_g, MAGIC); the merge
       masks sign(|x| - pred(median_g)) -> u8 (exact >= compare).
  DVE: per group the clamp w = min(max(x, A_g), B_g); the exact scale
       y_g = (t2 - MAGIC) * d_g; the predicated merge of group results.

Clamp-first + magic-number rounding reproduces the reference bit-exactly
except for round(x * (1/d)) vs round(x / d) boundary flips (~1e-6 of
elements, one quantization step each).
"""

import sys

for _p in ("/opt/trn_rl_repo", "/root/.axon_site/_ro/trn_rl_repo"):
    if _p not in sys.path:
        sys.path.append(_p)

import numpy as np

from concourse import bass, mybir

N_CORES = 8
FULL_SHAPE = (4, 4096, 4096)
TOTAL = FULL_SHAPE[0] * FULL_SHAPE[1] * FULL_SHAPE[2]  # 67108864
SHARD = TOTAL // N_CORES  # 8388608
P = 128
F = 4096
TILES = SHARD // (P * F)  # 16
G = 10
MAGIC = float(np.float32(1.5 * 2**23))  # add+sub rounds to nearest int (RNE)

f32 = mybir.dt.float32
u8 = mybir.dt.uint8
Alu = mybir.AluOpType
Act = mybir.ActivationFunctionType


def _f32(v) -> float:
    return float(np.float32(v))


def build_program(medians, deltas, zero_points, tiles=TILES):
    med = np.asarray(medians, dtype=np.float32)
    d = np.asarray(deltas, dtype=np.float32)
    zp = np.asarray(zero_points, dtype=np.float32)

    r = (np.float32(1.0) / d).astype(np.float32)
    A = (-zp * d).astype(np.float32)
    B = ((np.float32(255.0) - zp) * d).astype(np.float32)
    # |x| >= m  <=>  |x| > pred(m)  <=>  sigmoid(K*(|x| - pred(m))) rounds to 1
    # in u8 (K*ulp >= 2^15 so the sigmoid saturates exactly; the |x|==pred(m)
    # point gives sigmoid(0)=0.5 which rounds to u8 0 == mask false, correct).
    med_pred = np.nextafter(med, np.float32(-np.inf), dtype=np.float32)
    MASK_K = np.float32(2.0**40)

    nc = bass.Bass()
    xin = nc.declare_dram_parameter("x", [tiles, P, F], f32, isOutput=False)
    yout = nc.declare_dram_parameter("y", [tiles, P, F], f32, isOutput=True)

    # [P, 1] constant columns for ACT bias operands
    def const_col(name, val):
        t = nc.alloc_sbuf_tensor(name, [P, 1], f32)
        nc.gpsimd.memset(t.ap(), float(np.float32(val)))
        return t.ap()

    magic_ap = const_col("c_magic", MAGIC)
    medp_aps = [
        const_col(f"c_mp{i}", -(MASK_K * med_pred[i])) for i in range(G - 1)
    ]
    nc.all_engine_barrier()

    with (
        nc.Block() as block,
        nc.semaphore("s_ld0") as s_ld0,  # +16 per even-tile input DMA
        nc.semaphore("s_ld1") as s_ld1,  # +16 per odd-tile input DMA
        nc.semaphore("s_ab") as s_ab,  # +1 per |x| tile (ACT)
        nc.semaphore("s_cl") as s_cl,  # +1 per clamp (DVE), 10/tile
        nc.semaphore("s_rd") as s_rd,  # +1 per round (ACT), 10/tile
        nc.semaphore("s_sc") as s_sc,  # +1 per scale (DVE), 10/tile
        nc.semaphore("s_mk") as s_mk,  # +1 per mask (ACT), 9/tile
        nc.semaphore("s_cp") as s_cp,  # +1 per predicated copy (DVE), 9/tile
        nc.semaphore("s_st0") as s_st0,  # +16 per even-tile output DMA
        nc.semaphore("s_st1") as s_st1,  # +16 per odd-tile output DMA
        nc.sbuf_tensor("xt", [P, 2, F], f32) as xt,
        nc.sbuf_tensor("tb", [P, 2, F], f32) as tb,
        nc.sbuf_tensor("yy", [P, 2, F], f32) as yy,
        nc.sbuf_tensor("ww", [P, 2, F], f32) as ww,  # clamp out, by group parity
        nc.sbuf_tensor("t2", [P, 2, F], f32) as t2,  # round out, by group parity
        nc.sbuf_tensor("yg", [P, F], f32) as yg,
        nc.sbuf_tensor("mk", [P, 2, F], u8) as mk,  # masks, by group parity
    ):
        s_ld = (s_ld0, s_ld1)
        s_st = (s_st0, s_st1)

        def ld_val(t):
            return 16 * (t // 2 + 1)

        def st_val(t):
            return 16 * (t // 2 + 1)

        @block.sync
        def _(sync: bass.BassEngine):
            for t in range(tiles):
                s = t % 2
                if t >= 2:
                    # xt[:, s] free once tile t-2's clamps and |x| are done
                    sync.wait_ge(s_cl, G * (t - 1))
                    sync.wait_ge(s_ab, t - 1)
                sync.dma_start(out=xt[:, s], in_=xin[t]).then_inc(s_ld[s], 16)
                if t >= 1:
                    sync.wait_ge(s_cp, 9 * t)  # tile t-1 fully merged
                    sync.dma_start(out=yout[t - 1], in_=yy[:, (t - 1) % 2]).then_inc(
                        s_st[(t - 1) % 2], 16
                    )
            t = tiles - 1
            sync.wait_ge(s_cp, 9 * tiles)
            sync.dma_start(out=yout[t], in_=yy[:, t % 2]).then_inc(s_st[t % 2], 16)
            sync.wait_ge(s_st0, st_val(2 * ((tiles - 1) // 2)))
            sync.wait_ge(s_st1, st_val(2 * ((tiles - 2) // 2) + 1))

        @block.scalar
        def _(scalar: bass.BassEngine):
            for t in range(tiles):
                s = t % 2
                scalar.wait_ge(s_ld[s], ld_val(t))
                scalar.activation(out=tb[:, s], in_=xt[:, s], func=Act.Abs).then_inc(
                    s_ab, 1
                )
                scalar.drain()
                for g in range(G):
                    # t2[g%2] = w * r_g + MAGIC  (single-rounded fma)
                    if t > 0 or g >= 2:
                        scalar.wait_ge(s_sc, G * t + g - 1)  # t2 slot consumed
                    scalar.wait_ge(s_cl, G * t + g + 1)  # w ready
                    scalar.activation(
                        out=t2[:, g % 2], in_=ww[:, g % 2], func=Act.Identity,
                        bias=magic_ap, scale=_f32(r[g]),
                    ).then_inc(s_rd, 1)
                    if g < 9:
                        # mask for group g+1: sign(|x| - pred(med_g)) -> u8
                        if t > 0 or g >= 2:
                            scalar.wait_ge(s_cp, 9 * t + max(g - 1, 0))  # mk slot free
                        scalar.activation(
                            out=mk[:, (g + 1) % 2], in_=tb[:, s], func=Act.Sigmoid,
                            bias=medp_aps[g], scale=float(MASK_K),
                        ).then_inc(s_mk, 1)

        @block.vector
        def _(vector: bass.BassEngine):
            for t in range(tiles):
                s = t % 2
                vector.wait_ge(s_ld[s], ld_val(t))
                if t >= 2:
                    vector.wait_ge(s_st[s], st_val(t - 2))  # yy[:, s] stored
                for g in range(G):
                    # clamp for group g (skip wait: ACT round_{g-2} frees ww slot,
                    # implied by the s_rd wait of scale_{g-1} below for g>=2... but
                    # make it explicit for the race detector)
                    if t > 0 or g >= 2:
                        vector.wait_ge(s_rd, G * t + g - 1)  # ww slot consumed
                    vector.tensor_scalar(
                        out=ww[:, g % 2], in0=xt[:, s],
                        scalar1=_f32(A[g]), scalar2=_f32(B[g]),
                        op0=Alu.max, op1=Alu.min,
                    ).then_inc(s_cl, 1)
                    vector.drain()
                    # scale for group g: dst = (t2 - MAGIC) * d_g
                    dst = yy[:, s] if g == 0 else yg[:]
                    vector.wait_ge(s_rd, G * t + g + 1)
                    vector.tensor_scalar(
                        out=dst, in0=t2[:, g % 2],
                        scalar1=MAGIC, scalar2=_f32(d[g]),
                        op0=Alu.subtract, op1=Alu.mult,
                    ).then_inc(s_sc, 1)
                    vector.drain()
                    if g > 0:
                        vector.wait_ge(s_mk, 9 * t + g)
                        vector.copy_predicated(
                            out=yy[:, s], mask=mk[:, g % 2], data=yg[:]
                        ).then_inc(s_cp, 1)
                        vector.drain()

    return nc


def run(x, medians, deltas, zero_points, trace=False):
    from concourse.bass_utils import run_bass_kernel_spmd

    nc = build_program(medians, deltas, zero_points)

    xf = np.ascontiguousarray(np.asarray(x, dtype=np.float32)).reshape(-1)
    shards = [
        xf[i * SHARD : (i + 1) * SHARD].reshape(TILES, P, F) for i in range(N_CORES)
    ]
    in_maps = [{"x": s} for s in shards]
    res = run_bass_kernel_spmd(nc, in_maps, list(range(N_CORES)), trace=trace)
    out = np.concatenate(
        [res.results[i]["y"].reshape(-1) for i in range(N_CORES)]
    ).reshape(FULL_SHAPE)
    return out.astype(np.float32), res


def kernel(x, medians, deltas, zero_points):
    out, _ = run(x, medians, deltas, zero_points, trace=False)
    return out


# revision 14
# speedup vs baseline: 1.6246x; 1.4284x over previous
"""Trainium2 Bass kernel for nn_BERTMADActQuantizer.

y = (clip(round(x / d[grp]) + zp[grp], 0, 255) - zp[grp]) * d[grp]
where grp = clip(#(medians <= |x|), 0, 9).

Data-parallel shard of x across 8 NeuronCores (shapes hardcoded). Per core a
raw-bass double-buffered pipeline streams [128, F] tiles. Work is split
between the two elementwise engines:

  ACT: |x|; per group g the RNE round t2 = fma(w, 1/d_g, MAGIC); the merge
       masks sign(|x| - pred(median_g)) -> u8 (exact >= compare).
  DVE: per group the clamp w = min(max(x, A_g), B_g); the exact scale
       y_g = (t2 - MAGIC) * d_g; the predicated merge of group results.

Clamp-first + magic-number rounding reproduces the reference bit-exactly
except for round(x * (1/d)) vs round(x / d) boundary flips (~1e-6 of
elements, one quantization step each).
"""

import sys

for _p in ("/opt/trn_rl_repo", "/root/.axon_site/_ro/trn_rl_repo"):
    if _p not in sys.path:
        sys.path.append(_p)

import numpy as np

from concourse import bass, mybir

N_CORES = 8
FULL_SHAPE = (4, 4096, 4096)
TOTAL = FULL_SHAPE[0] * FULL_SHAPE[1] * FULL_SHAPE[2]  # 67108864
SHARD = TOTAL // N_CORES  # 8388608
P = 128
F = 4096
TILES = SHARD // (P * F)  # 16
G = 10
MAGIC = float(np.float32(1.5 * 2**23))  # add+sub rounds to nearest int (RNE)

f32 = mybir.dt.float32
u8 = mybir.dt.uint8
Alu = mybir.AluOpType
Act = mybir.ActivationFunctionType


def _f32(v) -> float:
    return float(np.float32(v))


def build_program(medians, deltas, zero_points, tiles=TILES):
    med = np.asarray(medians, dtype=np.float32)
    d = np.asarray(deltas, dtype=np.float32)
    zp = np.asarray(zero_points, dtype=np.float32)

    r = (np.float32(1.0) / d).astype(np.float32)
    A = (-zp * d).astype(np.float32)
    B = ((np.float32(255.0) - zp) * d).astype(np.float32)
    # |x| >= m  <=>  |x| > pred(m)  <=>  sigmoid(K*(|x| - pred(m))) rounds to 1
    # in u8 (K*ulp >= 2^15 so the sigmoid saturates exactly; the |x|==pred(m)
    # point gives sigmoid(0)=0.5 which rounds to u8 0 == mask false, correct).
    med_pred = np.nextafter(med, np.float32(-np.inf), dtype=np.float32)
    MASK_K = np.float32(2.0**40)

    nc = bass.Bass()
    xin = nc.declare_dram_parameter("x", [tiles, P, F], f32, isOutput=False)
    yout = nc.declare_dram_parameter("y", [tiles, P, F], f32, isOutput=True)

    # [P, 1] constant columns for ACT bias operands
    def const_col(name, val):
        t = nc.alloc_sbuf_tensor(name, [P, 1], f32)
        nc.gpsimd.memset(t.ap(), float(np.float32(val)))
        return t.ap()

    magic_ap = const_col("c_magic", MAGIC)
    medp_aps = [
        const_col(f"c_mp{i}", -(MASK_K * med_pred[i])) for i in range(G - 1)
    ]
    nc.all_engine_barrier()

    with (
        nc.Block() as block,
        nc.semaphore("s_ld0") as s_ld0,  # +16 per even-tile input DMA
        nc.semaphore("s_ld1") as s_ld1,  # +16 per odd-tile input DMA
        nc.semaphore("s_ab") as s_ab,  # +1 per |x| tile (ACT)
        nc.semaphore("s_cl") as s_cl,  # +1 per clamp (DVE), 10/tile
        nc.semaphore("s_rd") as s_rd,  # +1 per round (ACT), 10/tile
        nc.semaphore("s_sc") as s_sc,  # +1 per scale (DVE), 10/tile
        nc.semaphore("s_mk") as s_mk,  # +1 per mask (ACT), 9/tile
        nc.semaphore("s_cp") as s_cp,  # +1 per predicated copy (DVE), 9/tile
        nc.semaphore("s_st0") as s_st0,  # +16 per even-tile output DMA
        nc.semaphore("s_st1") as s_st1,  # +16 per odd-tile output DMA
        nc.sbuf_tensor("xt", [P, 2, F], f32) as xt,
        nc.sbuf_tensor("tb", [P, 2, F], f32) as tb,
        nc.sbuf_tensor("yy", [P, 2, F], f32) as yy,
        nc.sbuf_tensor("ww", [P, 4, F], f32) as ww,  # clamp+round ring, slot k%4
        nc.sbuf_tensor("yg", [P, F], f32) as yg,
        nc.sbuf_tensor("mk", [P, 4, F], u8) as mk,  # mask ring, slot j%4
    ):
        s_ld = (s_ld0, s_ld1)
        s_st = (s_st0, s_st1)

        def ld_val(t):
            return 16 * (t // 2 + 1)

        def st_val(t):
            return 16 * (t // 2 + 1)

        @block.sync
        def _(sync: bass.BassEngine):
            for t in range(tiles):
                s = t % 2
                if t >= 2:
                    # xt[:, s] free once tile t-2's clamps and |x| are done
                    sync.wait_ge(s_cl, G * (t - 1))
                    sync.wait_ge(s_ab, t - 1)
                sync.dma_start(out=xt[:, s], in_=xin[t]).then_inc(s_ld[s], 16)
                if t >= 1:
                    sync.wait_ge(s_cp, 9 * t)  # tile t-1 fully merged
                    sync.dma_start(out=yout[t - 1], in_=yy[:, (t - 1) % 2]).then_inc(
                        s_st[(t - 1) % 2], 16
                    )
            t = tiles - 1
            sync.wait_ge(s_cp, 9 * tiles)
            sync.dma_start(out=yout[t], in_=yy[:, t % 2]).then_inc(s_st[t % 2], 16)
            sync.wait_ge(s_st0, st_val(2 * ((tiles - 1) // 2)))
            sync.wait_ge(s_st1, st_val(2 * ((tiles - 2) // 2) + 1))

        @block.scalar
        def _(scalar: bass.BassEngine):
            for t in range(tiles):
                s = t % 2
                scalar.wait_ge(s_ld[s], ld_val(t))
                scalar.activation(out=tb[:, s], in_=xt[:, s], func=Act.Abs).then_inc(
                    s_ab, 1
                )
                scalar.drain()
                for g in range(G):
                    k = G * t + g  # global group index; ww slot = k % 4
                    # in-place round: ww[k%4] = ww[k%4] * r_g + MAGIC (fma, RNE)
                    scalar.wait_ge(s_cl, k + 1)  # clamp_k done
                    scalar.activation(
                        out=ww[:, k % 4], in_=ww[:, k % 4], func=Act.Identity,
                        bias=magic_ap, scale=_f32(r[g]),
                    ).then_inc(s_rd, 1)
                    if g < 9:
                        j = 9 * t + g  # global mask index; mk slot = j % 4
                        if j >= 4:
                            scalar.wait_ge(s_cp, j - 3)  # mk slot's cp consumed
                        scalar.activation(
                            out=mk[:, j % 4], in_=tb[:, s], func=Act.Sigmoid,
                            bias=medp_aps[g], scale=float(MASK_K),
                        ).then_inc(s_mk, 1)

        @block.vector
        def _(vector: bass.BassEngine):
            LOOKAHEAD = 4

            def emit_clamp(vector, t, g):
                k = G * t + g
                if k >= LOOKAHEAD:
                    vector.wait_ge(s_sc, k - (LOOKAHEAD - 1))  # ww slot free
                vector.tensor_scalar(
                    out=ww[:, k % 4], in0=xt[:, t % 2],
                    scalar1=_f32(A[g]), scalar2=_f32(B[g]),
                    op0=Alu.max, op1=Alu.min,
                ).then_inc(s_cl, 1)

            def emit_scale(vector, t, g):
                k = G * t + g
                dst = yy[:, t % 2] if g == 0 else yg[:]
                vector.wait_ge(s_rd, k + 1)  # round_k done
                vector.tensor_scalar(
                    out=dst, in0=ww[:, k % 4],
                    scalar1=MAGIC, scalar2=_f32(d[g]),
                    op0=Alu.subtract, op1=Alu.mult,
                ).then_inc(s_sc, 1)
                vector.drain()

            def emit_cp(vector, t, g):
                j = 9 * t + (g - 1)
                vector.wait_ge(s_mk, j + 1)
                vector.copy_predicated(
                    out=yy[:, t % 2], mask=mk[:, j % 4], data=yg[:]
                ).then_inc(s_cp, 1)
                vector.drain()

            for t in range(tiles):
                s = t % 2
                vector.wait_ge(s_ld[s], ld_val(t))
                if t >= 2:
                    vector.wait_ge(s_st[s], st_val(t - 2))  # yy[:, s] stored
                for g in range(min(LOOKAHEAD, G)):
                    emit_clamp(vector, t, g)
                for g in range(G):
                    emit_scale(vector, t, g)
                    if g + LOOKAHEAD < G:
                        emit_clamp(vector, t, g + LOOKAHEAD)
                    if g > 0:
                        emit_cp(vector, t, g)

    return nc


def run(x, medians, deltas, zero_points, trace=False):
    from concourse.bass_utils import run_bass_kernel_spmd

    nc = build_program(medians, deltas, zero_points)

    xf = np.ascontiguousarray(np.asarray(x, dtype=np.float32)).reshape(-1)
    shards = [
        xf[i * SHARD : (i + 1) * SHARD].reshape(TILES, P, F) for i in range(N_CORES)
    ]
    in_maps = [{"x": s} for s in shards]
    res = run_bass_kernel_spmd(nc, in_maps, list(range(N_CORES)), trace=trace)
    out = np.concatenate(
        [res.results[i]["y"].reshape(-1) for i in range(N_CORES)]
    ).reshape(FULL_SHAPE)
    return out.astype(np.float32), res


def kernel(x, medians, deltas, zero_points):
    out, _ = run(x, medians, deltas, zero_points, trace=False)
    return out


# revision 19
# speedup vs baseline: 1.6632x; 1.0237x over previous
"""Trainium2 Bass kernel for nn_BERTMADActQuantizer.

y = (clip(round(x / d[grp]) + zp[grp], 0, 255) - zp[grp]) * d[grp]
where grp = clip(#(medians <= |x|), 0, 9).

Data-parallel shard of x across 8 NeuronCores (shapes hardcoded). Per core a
raw-bass double-buffered pipeline streams [128, F] tiles. Work is split
between the two elementwise engines:

  ACT: |x|; per group g the RNE round t2 = fma(w, 1/d_g, MAGIC); the merge
       masks sign(|x| - pred(median_g)) -> u8 (exact >= compare).
  DVE: per group the clamp w = min(max(x, A_g), B_g); the exact scale
       y_g = (t2 - MAGIC) * d_g; the predicated merge of group results.

Clamp-first + magic-number rounding reproduces the reference bit-exactly
except for round(x * (1/d)) vs round(x / d) boundary flips (~1e-6 of
elements, one quantization step each).
"""

import sys

for _p in ("/opt/trn_rl_repo", "/root/.axon_site/_ro/trn_rl_repo"):
    if _p not in sys.path:
        sys.path.append(_p)

import numpy as np

from concourse import bass, mybir

N_CORES = 8
FULL_SHAPE = (4, 4096, 4096)
TOTAL = FULL_SHAPE[0] * FULL_SHAPE[1] * FULL_SHAPE[2]  # 67108864
SHARD = TOTAL // N_CORES  # 8388608
P = 128
F = 4096
TILES = SHARD // (P * F)  # 16
G = 10
MAGIC = float(np.float32(1.5 * 2**23))  # add+sub rounds to nearest int (RNE)

f32 = mybir.dt.float32
u8 = mybir.dt.uint8
Alu = mybir.AluOpType
Act = mybir.ActivationFunctionType


def _f32(v) -> float:
    return float(np.float32(v))


def build_program(medians, deltas, zero_points, tiles=TILES):
    med = np.asarray(medians, dtype=np.float32)
    d = np.asarray(deltas, dtype=np.float32)
    zp = np.asarray(zero_points, dtype=np.float32)

    r = (np.float32(1.0) / d).astype(np.float32)
    A = (-zp * d).astype(np.float32)
    B = ((np.float32(255.0) - zp) * d).astype(np.float32)
    # |x| >= m  <=>  |x| > pred(m)  <=>  sigmoid(K*(|x| - pred(m))) rounds to 1
    # in u8 (K*ulp >= 2^15 so the sigmoid saturates exactly; the |x|==pred(m)
    # point gives sigmoid(0)=0.5 which rounds to u8 0 == mask false, correct).
    med_pred = np.nextafter(med, np.float32(-np.inf), dtype=np.float32)
    MASK_K = np.float32(2.0**40)

    nc = bass.Bass()
    xin = nc.declare_dram_parameter("x", [tiles, P, F], f32, isOutput=False)
    yout = nc.declare_dram_parameter("y", [tiles, P, F], f32, isOutput=True)

    # [P, 1] constant columns for ACT bias operands
    def const_col(name, val):
        t = nc.alloc_sbuf_tensor(name, [P, 1], f32)
        nc.gpsimd.memset(t.ap(), float(np.float32(val)))
        return t.ap()

    magic_ap = const_col("c_magic", MAGIC)
    medp_aps = [
        const_col(f"c_mp{i}", -(MASK_K * med_pred[i])) for i in range(G - 1)
    ]
    # Group ACG runs its clamp on ACT as a relu pair:
    #   u = relu(x - A); w'' = relu((B - A) - u); t1 = C - r*w''
    # with C = MAGIC + 255 - zp (exact int). Equivalent up to a sub-ulp shift
    # of the round boundary (same error class as the reciprocal multiply).
    ACG = 5
    BA_ap = const_col("c_ba", np.float32(B[ACG] - A[ACG]))
    C_ap = const_col("c_C", np.float32(MAGIC + 255.0 - zp[ACG]))
    negA_ap = const_col("c_negA", np.float32(-A[ACG]))
    nc.all_engine_barrier()

    with (
        nc.Block() as block,
        nc.semaphore("s_ld0") as s_ld0,  # +16 per even-tile input DMA
        nc.semaphore("s_ld1") as s_ld1,  # +16 per odd-tile input DMA
        nc.semaphore("s_ab") as s_ab,  # +1 per |x| tile (ACT)
        nc.semaphore("s_cl") as s_cl,  # +1 per DVE clamp, 9/tile (ACG on ACT)
        nc.semaphore("s_rd") as s_rd,  # +1 per round (ACT), 10/tile
        nc.semaphore("s_sc") as s_sc,  # +1 per scale (DVE), 10/tile
        nc.semaphore("s_mk") as s_mk,  # +1 per mask (ACT), 9/tile
        nc.semaphore("s_cp") as s_cp,  # +1 per predicated copy (DVE), 9/tile
        nc.semaphore("s_st0") as s_st0,  # +16 per even-tile output DMA
        nc.semaphore("s_st1") as s_st1,  # +16 per odd-tile output DMA
        nc.sbuf_tensor("xt", [P, 2, F], f32) as xt,
        nc.sbuf_tensor("tb", [P, 2, F], f32) as tb,
        nc.sbuf_tensor("yy", [P, 2, F], f32) as yy,
        nc.sbuf_tensor("ww", [P, 4, F], f32) as ww,  # clamp+round ring, slot k%4
        nc.sbuf_tensor("yg", [P, F], f32) as yg,
        nc.sbuf_tensor("mk", [P, 4, F], u8) as mk,  # mask ring, slot j%4
    ):
        s_ld = (s_ld0, s_ld1)
        s_st = (s_st0, s_st1)

        def ld_val(t):
            return 16 * (t // 2 + 1)

        def st_val(t):
            return 16 * (t // 2 + 1)

        @block.sync
        def _(sync: bass.BassEngine):
            for t in range(tiles):
                s = t % 2
                if t >= 2:
                    # xt[:, s] free once tile t-2's rounds (imply clamps and the
                    # ACT relu-clamp) and |x| are done
                    sync.wait_ge(s_rd, G * (t - 1))
                    sync.wait_ge(s_ab, t - 1)
                sync.dma_start(out=xt[:, s], in_=xin[t]).then_inc(s_ld[s], 16)
                if t >= 1:
                    sync.wait_ge(s_cp, 9 * t)  # tile t-1 fully merged
                    sync.dma_start(out=yout[t - 1], in_=yy[:, (t - 1) % 2]).then_inc(
                        s_st[(t - 1) % 2], 16
                    )
            t = tiles - 1
            sync.wait_ge(s_cp, 9 * tiles)
            sync.dma_start(out=yout[t], in_=yy[:, t % 2]).then_inc(s_st[t % 2], 16)
            sync.wait_ge(s_st0, st_val(2 * ((tiles - 1) // 2)))
            sync.wait_ge(s_st1, st_val(2 * ((tiles - 2) // 2) + 1))

        @block.scalar
        def _(scalar: bass.BassEngine):
            for t in range(tiles):
                s = t % 2
                scalar.wait_ge(s_ld[s], ld_val(t))
                scalar.activation(out=tb[:, s], in_=xt[:, s], func=Act.Abs).then_inc(
                    s_ab, 1
                )
                scalar.drain()
                for g in range(G):
                    k = G * t + g  # global group index; ww slot = k % 4
                    if g == ACG:
                        # full clamp+round on ACT (relu pair + fma)
                        if k >= 4:
                            scalar.wait_ge(s_sc, k - 3)  # ww slot free
                        scalar.activation(
                            out=ww[:, k % 4], in_=xt[:, s], func=Act.Relu,
                            bias=negA_ap,
                        )
                        scalar.drain()
                        scalar.activation(
                            out=ww[:, k % 4], in_=ww[:, k % 4], func=Act.Relu,
                            bias=BA_ap, scale=-1.0,
                        )
                        scalar.drain()
                        scalar.activation(
                            out=ww[:, k % 4], in_=ww[:, k % 4], func=Act.Identity,
                            bias=C_ap, scale=_f32(-r[ACG]),
                        ).then_inc(s_rd, 1)
                    else:
                        # in-place round: ww[k%4] = ww[k%4]*r_g + MAGIC (fma, RNE)
                        # placeholder (9 per tile, ACG skipped)
                        dve_ord = 9 * t + (g + 1 if g < ACG else g)
                        scalar.wait_ge(s_cl, dve_ord)  # clamp_k done
                        scalar.activation(
                            out=ww[:, k % 4], in_=ww[:, k % 4], func=Act.Identity,
                            bias=magic_ap, scale=_f32(r[g]),
                        ).then_inc(s_rd, 1)
                    if g < 9:
                        j = 9 * t + g  # global mask index; mk slot = j % 4
                        if j >= 4:
                            scalar.wait_ge(s_cp, j - 3)  # mk slot's cp consumed
                        scalar.activation(
                            out=mk[:, j % 4], in_=tb[:, s], func=Act.Sigmoid,
                            bias=medp_aps[g], scale=float(MASK_K),
                        ).then_inc(s_mk, 1)

        @block.vector
        def _(vector: bass.BassEngine):
            LOOKAHEAD = 4
            NT = tiles * G

            def emit_clamp(vector, k):
                t, g = divmod(k, G)
                if g == 0:
                    vector.wait_ge(s_ld[t % 2], ld_val(t))  # xt[t] loaded
                if g == ACG:
                    return  # ACT computes this group's clamp+round
                if k >= LOOKAHEAD:
                    vector.wait_ge(s_sc, k - (LOOKAHEAD - 1))  # ww slot free
                vector.tensor_scalar(
                    out=ww[:, k % 4], in0=xt[:, t % 2],
                    scalar1=_f32(A[g]), scalar2=_f32(B[g]),
                    op0=Alu.max, op1=Alu.min,
                ).then_inc(s_cl, 1)

            def emit_scale(vector, k):
                t, g = divmod(k, G)
                if g == 0 and t >= 2:
                    vector.wait_ge(s_st[t % 2], st_val(t - 2))  # yy slot stored
                dst = yy[:, t % 2] if g == 0 else yg[:]
                vector.wait_ge(s_rd, k + 1)  # round_k done
                vector.tensor_scalar(
                    out=dst, in0=ww[:, k % 4],
                    scalar1=MAGIC, scalar2=_f32(d[g]),
                    op0=Alu.subtract, op1=Alu.mult,
                ).then_inc(s_sc, 1)
                vector.drain()

            def emit_cp(vector, k):
                t, g = divmod(k, G)
                j = 9 * t + (g - 1)
                vector.wait_ge(s_mk, j + 1)
                vector.copy_predicated(
                    out=yy[:, t % 2], mask=mk[:, j % 4], data=yg[:]
                ).then_inc(s_cp, 1)
                vector.drain()

            for k in range(LOOKAHEAD):
                emit_clamp(vector, k)
            for k in range(NT):
                emit_scale(vector, k)
                if k + LOOKAHEAD < NT:
                    emit_clamp(vector, k + LOOKAHEAD)
                if k % G > 0:
                    emit_cp(vector, k)

    return nc


def run(x, medians, deltas, zero_points, trace=False):
    from concourse.bass_utils import run_bass_kernel_spmd

    nc = build_program(medians, deltas, zero_points)

    xf = np.ascontiguousarray(np.asarray(x, dtype=np.float32)).reshape(-1)
    shards = [
        xf[i * SHARD : (i + 1) * SHARD].reshape(TILES, P, F) for i in range(N_CORES)
    ]
    in_maps = [{"x": s} for s in shards]
    res = run_bass_kernel_spmd(nc, in_maps, list(range(N_CORES)), trace=trace)
    out = np.concatenate(
        [res.results[i]["y"].reshape(-1) for i in range(N_CORES)]
    ).reshape(FULL_SHAPE)
    return out.astype(np.float32), res


def kernel(x, medians, deltas, zero_points):
    out, _ = run(x, medians, deltas, zero_points, trace=False)
    return out
